# revision 39
# baseline (speedup 1.0000x reference)
"""ConvolutionKAN Trainium2 kernel (8-core SPMD, data-parallel over batch).

Math: the reference computes, per conv patch element x (one of 3x3x32 = 288
taps x channels) a cubic B-spline basis beta_0..7(x) on a uniform grid over
[-1, 1], contracts with (spline_kernel * scale_factor), and adds a
silu(x) @ scale_factor term plus bias.

Key identity used here: on the uniform grid, with t = 2.5 x + 2.5 in [0, 5),

    beta_j(x) = (1/6) sum_{i=0..4} (-1)^i C(4,i) relu(t - (j + i - 3))^3

For shift m <= 0 the relu never clips on t >= 0, so those terms are plain
cubics in x; terms with m >= 5 are identically zero on t < 5.  For m = 1, 2
the REFLECTED split relu(t-m)^3 = (t-m)^3 + relu(m-t)^3 folds the cubic into
the polynomial part, keeping every on-device feature bounded by 8.

Per-element features (8):  [x, x^2, x^3, S1, S2, R3, R4, silu(x)] where
S_m = relu(m - t)^3, R_m = relu(t - m)^3.  Folding the basis->feature linear
map into the weights host-side turns the KAN conv into, per conv tap, a
dense fp32r matmul with K = 8*32 = 256 -- the constant row collapses into
the bias.  Per core: compute the 8 features per input pixel, transpose to
[K, pixels] via the PE, then 9 taps x 2 K-chunks of 128 rows accumulate into
PSUM over windows of 4 output rows (N = 496 columns).

Perf notes (from HW traces):
- fp32r matmuls stream ~1.1 cycles/col and get FASTER as the PE's DVFS
  ramps (387 -> 244 -> 230 ns per 496-col matmul over the run); bf16 pays a
  fixed shadow-buffer->array weight-fill (~128 cycles) per matmul and is
  NET SLOWER (253 ns flat).  So the GEMM stays fp32r, and the boot-time PE
  idle (~7 us) is filled with dummy fp32 matmuls purely to pre-ramp the
  clock before the real stream starts.
- Walrus rejects mixed 32/non-32-bit matmul inputs, so the transpose
  identity stays fp32r (1.5 cycles/row).  (The DMA XBAR transpose was
  measured 15x slower than its cost model; DVE transpose cannot cross
  partitions.)
- The 4 row-transposes of one chunk batch into a single PSUM bank
  (start=True zeroes the whole 2KB region; later rows accumulate onto
  pending-zero bytes); one scalar-engine copy drains all 4 (the DVE pays
  a much larger PSUM access latency).
- The feature relus run on the VECTOR engine (fused mul-add + in-place
  max), so the scalar engine's boot-time ACT-table loads never gate the
  pipeline and the Relu table is never loaded at all.
- Matmuls are ordered di=0 first and phase_b lags the transpose phases
  by two groups, so drain copies always land before their consumers.
- 62 output rows = 14x4 + 2x3: the last two groups are 3 rows (N = 372,
  still >= the 256 fp32r needs for 1 cycle/col), so no rows recompute.
- Boot DMAs spread across queues: x4_0 images split sync/scalar, x4_1
  on gpsimd SWDGE, weights in 3 SWDGE chunks.  Remaining ramp-in cost is
  dominated by DVFS clock-transition pauses (~0.8 us each), which are
  insensitive to scheduling.
"""

import numpy as np
from math import comb

KH = KW = 3
C = 32
FILTERS = 128
B, H, W = 16, 64, 64
OH = OW = 62
IN_SIZE = KH * KW * C  # 288
NCORES = 8
BLOC = B // NCORES  # 2 images per core

_FEATURE_ROWS = 8  # x, x2, x3, S1, S2, R3, R4, silu
_NTAP = KH * KW  # 9
_NCHUNK = 2  # 256 k-rows per tap -> 2 chunks of 128

_program_cache = {}
_SILU_NAME = "Silu"  # sim_test.py overrides: CoreSim lacks Silu


def _basis_row_map():
    """beta_j = sum_rc Bmat[j, rc] * feature_rc(x) + Bconst[j].

    Feature classes rc: 0:x 1:x^2 2:x^3 3:S1 4:S2 5:R3 6:R4 (silu handled
    separately).  S_m = relu(m - t)^3, R_m = relu(t - m)^3, t = 2.5 x + 2.5.
    """
    Bmat = np.zeros((8, 7), dtype=np.float64)
    Bconst = np.zeros((8,), dtype=np.float64)
    for j in range(8):
        for i in range(5):
            m = j + i - 3
            if m >= 5:
                continue
            cf = (-1) ** i * comb(4, i) / 6.0
            if m <= 2:
                # polynomial part (2.5 x + d)^3, d = 2.5 - m
                d = 2.5 - m
                Bmat[j, 2] += cf * 2.5**3
                Bmat[j, 1] += cf * 3 * 2.5**2 * d
                Bmat[j, 0] += cf * 3 * 2.5 * d * d
                Bconst[j] += cf * d**3
                if m in (1, 2):
                    Bmat[j, 2 + m] += cf  # S1 at col 3, S2 at col 4
            else:
                Bmat[j, 2 + m] += cf  # R3 at col 5, R4 at col 6
    return Bmat, Bconst


def _prep_weights(spline_kernel, scale_factor, bias):
    """Returns (wpk [128, 18, 128] fp32, bias_eff [128, 1] fp32).

    wpk[krow, tap*2 + q, o]: krow = (rc - 4*q)*32 + c for feature class rc
    (0..7, 7 = silu), chunk q = rc // 4, tap = di*3 + dj, c = channel.
    """
    Bmat, Bconst = _basis_row_map()
    sk = spline_kernel.astype(np.float64)  # (288, 8, 128)
    sf = scale_factor.astype(np.float64)  # (288, 128)
    w = sk * sf[:, None, :]  # (288, 8, 128)

    # (288, 7, 128): per input element, weight of each feature class
    wrows = np.einsum("jr,ijo->iro", Bmat, w)
    wfull = np.concatenate([wrows, sf[:, None, :]], axis=1)  # (288, 8, 128)
    # -> [tap, c, rc, o] -> [tap, rc, c, o]
    wfull = wfull.reshape(_NTAP, C, _FEATURE_ROWS, FILTERS).transpose(0, 2, 1, 3)
    # krow-major layout [128 krow, 9*2 tapchunk, 128 o]
    wpk = np.zeros((128, _NTAP * 2, FILTERS), dtype=np.float64)
    for tap in range(_NTAP):
        di, dj = divmod(tap, KW)
        for rc in range(_FEATURE_ROWS):
            q, rloc = divmod(rc, 4)
            wpk[rloc * 32 : (rloc + 1) * 32, (di * 2 + q) * 3 + dj, :] = wfull[tap, rc]

    bias_eff = bias.astype(np.float64) + np.einsum("j,ijo->o", Bconst, w)
    return (
        np.ascontiguousarray(wpk, dtype=np.float32),
        np.ascontiguousarray(bias_eff[:, None], dtype=np.float32),
    )


def _features_np(x):
    """Per-element features, fp32, matching the device computation.
    x: (..., ) -> (..., 8)"""
    x = x.astype(np.float32)
    feats = [x, x * x, (x * x) * x]
    for sc, b in ((-2.5, -1.5), (-2.5, -0.5), (2.5, -0.5), (2.5, -1.5)):
        v = np.maximum(np.float32(sc) * x + np.float32(b), np.float32(0.0))
        feats.append((v * v) * v)
    sig = 1.0 / (1.0 + np.exp(-x.astype(np.float64)))
    feats.append((x.astype(np.float64) * sig).astype(np.float32))
    return np.stack(feats, axis=-1)


def reference_sim(inputs, spline_kernel, scale_factor, bias, grid=None):
    """Host numpy simulation of the kernel math (for validation)."""
    wpk, bias_eff = _prep_weights(spline_kernel, scale_factor, bias)
    xb = inputs.astype(np.float32)
    feats = _features_np(xb).astype(np.float64)  # (B, H, W, 32, 8)
    out = np.zeros((xb.shape[0], OH, OW, FILTERS), dtype=np.float64)
    for di in range(KH):
        for dj in range(KW):
            tap = di * 3 + dj
            f = feats[:, di : di + OH, dj : dj + OW]  # (B, OH, OW, 32, 8)
            for q in range(2):
                wq = wpk[:, (di * 2 + q) * 3 + dj, :].astype(np.float64)
                # krow = rloc*32 + c, rc = q*4 + rloc
                fq = f[..., :, q * 4 : (q + 1) * 4]  # (..., 32, 4) c, rloc
                fq = np.moveaxis(fq, -1, -2).reshape(*f.shape[:3], 128)
                out += fq @ wq
    return (out + bias_eff[:, 0]).astype(np.float32)


def _build_program():
    import concourse.mybir as mybir
    from concourse import bacc
    from concourse.tile import TileContext
    from concourse.masks import make_identity

    FP = mybir.dt.float32
    FPR = mybir.dt.float32r
    BF = mybir.dt.bfloat16
    AF = mybir.ActivationFunctionType

    nc = bacc.Bacc()
    x_d = nc.dram_tensor("x", [BLOC, H, W, C], FP, kind="ExternalInput")
    w_d = nc.dram_tensor("wpk", [128, _NTAP * 2, FILTERS], FPR, kind="ExternalInput")
    b_d = nc.dram_tensor("bias_eff", [128, 1], FP, kind="ExternalInput")
    o_d = nc.dram_tensor("out", [128, OH, BLOC, OW], FP, kind="ExternalOutput")

    with TileContext(nc) as tc:
        with (
            tc.tile_pool(name="singles", bufs=1) as singles,
            tc.tile_pool(name="xp", bufs=3) as xp,
            tc.tile_pool(name="bp", bufs=3) as bp,
            tc.tile_pool(name="vp", bufs=2) as vp,
            tc.tile_pool(name="op", bufs=2) as op,
            tc.tile_pool(name="pt", bufs=4, space="PSUM") as pt,
            tc.tile_pool(name="po", bufs=2, space="PSUM") as po,
        ):
            # group-0 x loads go first so the sync DMA queue starts them
            # during boot (everything else below can overlap them)
            x4_0 = xp.tile([128, 4, C], FP, name="x4_0", tag="x4")
            for im in range(BLOC):
                src0 = x_d[im, 0:4, :, :].rearrange("r x c -> x r c")
                # image 0 on sync, image 1 on the scalar HWDGE queue: the two
                # descriptor gens run in parallel, landing x4_0 ~1.2us sooner
                # (it gates the whole feats(0) -> T(0) -> copies -> B(0) boot
                # chain)
                deng0 = nc.sync if im == 0 else nc.scalar
                deng0.dma_start(out=x4_0[im * 64 : (im + 1) * 64, :, :], in_=src0)

            ident = singles.tile([128, 128], FP)
            make_identity(nc, ident)
            identr = singles.tile([128, 128], FPR)
            nc.vector.tensor_copy(identr, ident)
            rbias = singles.tile([128, 2], FP)
            nc.gpsimd.memset(rbias[:, 0:1], -1.5)
            nc.gpsimd.memset(rbias[:, 1:2], -0.5)
            # pre-warm the Silu ACT table so its ~1.5us load happens during
            # boot (the relus moved to vector ops, so the Relu table is never
            # loaded at all)
            warm = singles.tile([128, 1], FP)
            nc.scalar.activation(warm, rbias[:, 0:1], getattr(AF, _SILU_NAME))

            # PE pre-heat: the Tensor engine's DVFS ramp means cold matmuls
            # run ~1.5x slower; burn ~5 us of zero matmuls during boot so the
            # clock is ramped when the real stream starts.  fp32 (4 cyc/col)
            # is used to get long-running instructions from few issues.
            zpre = singles.tile([128, 512], FP)
            nc.gpsimd.memset(zpre.rearrange("p a -> p a"), 0.0)
            zps = po.tile([128, 512], FP, name="zps", tag="ps")
            for i in range(3):
                nc.tensor.matmul(
                    zps, zpre[:, 0:128], zpre, start=True, stop=True
                )

            wt = singles.tile([128, _NTAP * 2, FILTERS], FPR)
            biasT = singles.tile([128, 1], FP)
            # feature-transpose buffers: [krow 128, row 64, img 2, x 64]
            bt0 = singles.tile([128, H, BLOC, 64], FPR)
            bt1 = singles.tile([128, H, BLOC, 64], FPR)
            bts = [bt0, bt1]

            # Phase FEAT (per group of 4 input rows): compute the 8 features
            # per pixel in [pixel, feature*32+c] layout.  Relus are emitted
            # before silu so the chunk-0 cube chain starts as early as
            # possible on the scalar queue.
            def phase_feat(g):
                if g == 0:
                    x4 = x4_0
                else:
                    x4 = xp.tile([128, 4, C], FP, name=f"x4_{g}", tag="x4")
                    # group 1 rides the gpsimd SWDGE queue: on the sync queue
                    # it lands ~4us late at boot and stalls the scalar-queue
                    # drain copies (and with them the first phase_b) behind
                    # silu(1)
                    deng = nc.gpsimd if g == 1 else nc.sync
                    for im in range(BLOC):
                        src = x_d[im, g * 4 : (g + 1) * 4, :, :].rearrange(
                            "r x c -> x r c"
                        )
                        deng.dma_start(
                            out=x4[im * 64 : (im + 1) * 64, :, :], in_=src
                        )
                b4 = bp.tile([128, 4, 256], FPR, name=f"b4_{g}", tag="b4")
                x2t = vp.tile([128, 4, C], FP, name=f"x2t_{g}", tag="x2t")
                V = vp.tile([128, 4, 128], FP, name=f"V_{g}", tag="V")
                V2 = vp.tile([128, 4, 128], FP, name=f"V2_{g}", tag="V2")

                # relus on VECTOR (fused mul-add then in-place max) so the
                # scalar engine's boot-time ACT-table chain never gates the
                # feature pipeline: S1 = relu(-2.5x - 1.5)^3, S2 = relu(-2.5x
                # - 0.5)^3, R3 = relu(2.5x - 0.5)^3, R4 = relu(2.5x - 1.5)^3.
                # S1 block first so the chunk-0 transpose starts early.
                AL = mybir.AluOpType
                nc.vector.tensor_scalar(V[:, :, 0:32], x4, -2.5, -1.5, AL.mult, AL.add)
                nc.vector.tensor_scalar_max(V[:, :, 0:32], V[:, :, 0:32], 0.0)
                nc.vector.tensor_mul(V2[:, :, 0:32], V[:, :, 0:32], V[:, :, 0:32])
                nc.vector.tensor_mul(b4[:, :, 96:128], V2[:, :, 0:32], V[:, :, 0:32])
                nc.vector.tensor_copy(b4[:, :, 0:32], x4)  # x
                nc.vector.tensor_mul(x2t, x4, x4)
                nc.vector.tensor_copy(b4[:, :, 32:64], x2t)  # x^2
                nc.vector.tensor_mul(b4[:, :, 64:96], x2t, x4)  # x^3
                for i, (sc, bv) in enumerate(
                    ((-2.5, -0.5), (2.5, -0.5), (2.5, -1.5)), start=1
                ):
                    nc.vector.tensor_scalar(
                        V[:, :, i * 32 : (i + 1) * 32], x4, sc, bv, AL.mult, AL.add
                    )
                nc.vector.tensor_scalar_max(V[:, :, 32:128], V[:, :, 32:128], 0.0)
                nc.vector.tensor_mul(V2[:, :, 32:128], V[:, :, 32:128], V[:, :, 32:128])
                nc.vector.tensor_mul(b4[:, :, 128:224], V2[:, :, 32:128], V[:, :, 32:128])
                nc.scalar.activation(b4[:, :, 224:256], x4, getattr(AF, _SILU_NAME))
                return b4

            # Phase T (per group): PE-transpose b4 into bt0/bt1.  The 4 rows
            # of one chunk batch into a single PSUM bank tile (start=True
            # zeroes the whole 2KB region; later rows accumulate onto
            # pending-zero bytes), then one vector copy moves all 4 rows out.
            def phase_t(g, b4):
                for q in range(2):
                    ptile = pt.tile([128, 4, 128], FPR, name=f"pt_{g}_{q}", tag="pt")
                    for r in range(4):
                        nc.tensor.matmul(
                            ptile[:, r, :],
                            b4[:, r, q * 128 : (q + 1) * 128],
                            identr,
                            is_transpose=True,
                            start=(r == 0),
                            stop=(r == 3),
                            skip_group_check=True,
                        )
                    # drain on the scalar engine: vector is loaded with the
                    # feature relus/cubes, scalar only has silu + bias now
                    dst = bts[q][:, g * 4 : (g + 1) * 4]
                    nc.scalar.copy(dst.rearrange("p r i x -> p (r i x)"),
                                   ptile.rearrange("p r x -> p (r x)"))

            # Phase B (per group of 4 output rows, N = 4*124 = 496 columns).
            # di = 0 matmuls only need rows from phase_t(og) (2 groups back),
            # so they run while the copies of phase_t(og+1)'s rows drain; the
            # di = 1, 2 matmuls then find their rows ready.  62 = 14*4 + 2*3:
            # the last two groups are 3 rows (N = 372, still >= the 256 fp32r
            # needs for 1 col/cycle) so no rows are recomputed.
            def phase_b(og):
                if og <= 13:
                    y0, nr = og * 4, 4
                elif og == 14:
                    y0, nr = 56, 3
                else:
                    y0, nr = 59, 3
                ps = po.tile([128, nr, 124], FP, name=f"ps_{og}", tag="ps")
                idx = 0
                for di in range(KH):
                    for q in range(2):
                        for dj in range(KW):
                            rhs = bts[q][:, y0 + di : y0 + di + nr, :, dj : dj + 62]
                            nc.tensor.matmul(
                                ps,
                                wt[:, (di * 2 + q) * 3 + dj, :],
                                rhs,
                                start=(idx == 0),
                                stop=(idx == 17),
                            )
                            idx += 1
                ot = op.tile([128, nr, 124], FP, name=f"ot_{og}", tag="ot")
                nc.scalar.activation(
                    ot,
                    ps,
                    AF.Identity,
                    bias=biasT[:, 0:1],
                    scale=1.0,
                )
                nc.sync.dma_start(out=o_d[:, y0 : y0 + nr, :, :], in_=ot)

            b4s = {0: phase_feat(0)}
            # Identity-table warm after phase_feat(0)'s scalar ops
            nc.scalar.activation(warm, rbias[:, 0:1], AF.Identity, bias=rbias[:, 0:1], scale=1.0)
            # The weight load is chunked on the gpsimd SWDGE queue (one big
            # SWDGE transfer would stall behind its descriptor generation)
            for wch in range(3):
                nc.gpsimd.dma_start(
                    out=wt[:, wch * 6 : (wch + 1) * 6, :],
                    in_=w_d[:, wch * 6 : (wch + 1) * 6, :],
                )
            nc.gpsimd.dma_start(out=biasT, in_=b_d[:, :])
            b4s[1] = phase_feat(1)
            phase_t(0, b4s.pop(0))
            b4s[2] = phase_feat(2)
            phase_t(1, b4s.pop(1))
            # phase_b lags phase_t by TWO groups: during pipeline fill the
            # scalar drain-copy chain (silu -> copies -> bias) hasn't
            # amortized yet, and a one-group lag leaves the PE stalling
            # ~0.8us per early group (each stall also resets the DVFS ramp)
            for g in range(2, H // 4):
                phase_t(g, b4s.pop(g))
                if g + 1 < H // 4:
                    b4s[g + 1] = phase_feat(g + 1)
                phase_b(g - 2)
            phase_b(14)
            phase_b(15)
    nc.compile()
    return nc


def _get_program():
    if "nc" not in _program_cache:
        _program_cache["nc"] = _build_program()
    return _program_cache["nc"]


def run_cores(inputs, spline_kernel, scale_factor, bias, trace=False):
    """Run the SPMD kernel on 8 cores; returns (out, BassKernelResults)."""
    from concourse.bass_utils import run_bass_kernel_spmd

    wpk, bias_eff = _prep_weights(spline_kernel, scale_factor, bias)
    x = np.ascontiguousarray(inputs, dtype=np.float32)
    in_maps = [
        {
            "x": x[i * BLOC : (i + 1) * BLOC],
            "wpk": wpk,
            "bias_eff": bias_eff,
        }
        for i in range(NCORES)
    ]
    nc = _get_program()
    res = run_bass_kernel_spmd(nc, in_maps, list(range(NCORES)), trace=trace)
    out = np.empty((B, OH, OW, FILTERS), dtype=np.float32)
    for i in range(NCORES):
        oc = res.results[i]["out"]  # [128, OH, BLOC, OW]
        out[i * BLOC : (i + 1) * BLOC] = np.transpose(oc, (2, 1, 3, 0))
    return out, res


def kernel(inputs, spline_kernel, scale_factor, bias, grid=None, **_):
    out, _res = run_cores(inputs, spline_kernel, scale_factor, bias, trace=False)
    return out


# revision 41
# speedup vs baseline: 1.0000x; 1.0000x over previous
"""ConvolutionKAN Trainium2 kernel (8-core SPMD, data-parallel over batch).

Math: the reference computes, per conv patch element x (one of 3x3x32 = 288
taps x channels) a cubic B-spline basis beta_0..7(x) on a uniform grid over
[-1, 1], contracts with (spline_kernel * scale_factor), and adds a
silu(x) @ scale_factor term plus bias.

Key identity used here: on the uniform grid, with t = 2.5 x + 2.5 in [0, 5),

    beta_j(x) = (1/6) sum_{i=0..4} (-1)^i C(4,i) relu(t - (j + i - 3))^3

For shift m <= 0 the relu never clips on t >= 0, so those terms are plain
cubics in x; terms with m >= 5 are identically zero on t < 5.  For m = 1, 2
the REFLECTED split relu(t-m)^3 = (t-m)^3 + relu(m-t)^3 folds the cubic into
the polynomial part, keeping every on-device feature bounded by 8.

Per-element features (8):  [x, x^2, x^3, S1, S2, R3, R4, silu(x)] where
S_m = relu(m - t)^3, R_m = relu(t - m)^3.  Folding the basis->feature linear
map into the weights host-side turns the KAN conv into, per conv tap, a
dense fp32r matmul with K = 8*32 = 256 -- the constant row collapses into
the bias.  Per core: compute the 8 features per input pixel, transpose to
[K, pixels] via the PE, then 9 taps x 2 K-chunks of 128 rows accumulate into
PSUM over windows of 4 output rows (N = 496 columns).

Perf notes (from HW traces):
- fp32r matmuls stream ~1.1 cycles/col and get FASTER as the PE's DVFS
  ramps (387 -> 244 -> 230 ns per 496-col matmul over the run); bf16 pays a
  fixed shadow-buffer->array weight-fill (~128 cycles) per matmul and is
  NET SLOWER (253 ns flat).  So the GEMM stays fp32r, and the boot-time PE
  idle (~7 us) is filled with dummy fp32 matmuls purely to pre-ramp the
  clock before the real stream starts.
- Walrus rejects mixed 32/non-32-bit matmul inputs, so the transpose
  identity stays fp32r (1.5 cycles/row).  (The DMA XBAR transpose was
  measured 15x slower than its cost model; DVE transpose cannot cross
  partitions.)
- The 4 row-transposes of one chunk batch into a single PSUM bank
  (start=True zeroes the whole 2KB region; later rows accumulate onto
  pending-zero bytes); one scalar-engine copy drains all 4 (the DVE pays
  a much larger PSUM access latency).
- The feature relus run on the VECTOR engine (fused mul-add + in-place
  max), so the scalar engine's boot-time ACT-table loads never gate the
  pipeline and the Relu table is never loaded at all.
- Matmuls are ordered di=0 first and phase_b lags the transpose phases
  by two groups, so drain copies always land before their consumers.
- 62 output rows = 14x4 + 2x3: the last two groups are 3 rows (N = 372,
  still >= the 256 fp32r needs for 1 cycle/col), so no rows recompute.
- Boot DMAs spread across queues: x4_0 images split sync/scalar, x4_1
  on gpsimd SWDGE, weights in 3 SWDGE chunks.  Remaining ramp-in cost is
  dominated by DVFS clock-transition pauses (~0.8 us each), which are
  insensitive to scheduling.
"""

import numpy as np
from math import comb

KH = KW = 3
C = 32
FILTERS = 128
B, H, W = 16, 64, 64
OH = OW = 62
IN_SIZE = KH * KW * C  # 288
NCORES = 8
BLOC = B // NCORES  # 2 images per core

_FEATURE_ROWS = 8  # x, x2, x3, S1, S2, R3, R4, silu
_NTAP = KH * KW  # 9
_NCHUNK = 2  # 256 k-rows per tap -> 2 chunks of 128

_program_cache = {}
_SILU_NAME = "Silu"  # sim_test.py overrides: CoreSim lacks Silu


def _basis_row_map():
    """beta_j = sum_rc Bmat[j, rc] * feature_rc(x) + Bconst[j].

    Feature classes rc: 0:x 1:x^2 2:x^3 3:S1 4:S2 5:R3 6:R4 (silu handled
    separately).  S_m = relu(m - t)^3, R_m = relu(t - m)^3, t = 2.5 x + 2.5.
    """
    Bmat = np.zeros((8, 7), dtype=np.float64)
    Bconst = np.zeros((8,), dtype=np.float64)
    for j in range(8):
        for i in range(5):
            m = j + i - 3
            if m >= 5:
                continue
            cf = (-1) ** i * comb(4, i) / 6.0
            if m <= 2:
                # polynomial part (2.5 x + d)^3, d = 2.5 - m
                d = 2.5 - m
                Bmat[j, 2] += cf * 2.5**3
                Bmat[j, 1] += cf * 3 * 2.5**2 * d
                Bmat[j, 0] += cf * 3 * 2.5 * d * d
                Bconst[j] += cf * d**3
                if m in (1, 2):
                    Bmat[j, 2 + m] += cf  # S1 at col 3, S2 at col 4
            else:
                Bmat[j, 2 + m] += cf  # R3 at col 5, R4 at col 6
    return Bmat, Bconst


def _prep_weights(spline_kernel, scale_factor, bias):
    """Returns (wpk [128, 18, 128] fp32, bias_eff [128, 1] fp32).

    wpk[krow, tap*2 + q, o]: krow = (rc - 4*q)*32 + c for feature class rc
    (0..7, 7 = silu), chunk q = rc // 4, tap = di*3 + dj, c = channel.
    """
    Bmat, Bconst = _basis_row_map()
    sk = spline_kernel.astype(np.float64)  # (288, 8, 128)
    sf = scale_factor.astype(np.float64)  # (288, 128)
    w = sk * sf[:, None, :]  # (288, 8, 128)

    # (288, 7, 128): per input element, weight of each feature class
    wrows = np.einsum("jr,ijo->iro", Bmat, w)
    wfull = np.concatenate([wrows, sf[:, None, :]], axis=1)  # (288, 8, 128)
    # -> [tap, c, rc, o] -> [tap, rc, c, o]
    wfull = wfull.reshape(_NTAP, C, _FEATURE_ROWS, FILTERS).transpose(0, 2, 1, 3)
    # krow-major layout [128 krow, 9*2 tapchunk, 128 o]
    wpk = np.zeros((128, _NTAP * 2, FILTERS), dtype=np.float64)
    for tap in range(_NTAP):
        for rc in range(_FEATURE_ROWS):
            q, rloc = divmod(rc, 4)
            wpk[rloc * 32 : (rloc + 1) * 32, tap * 2 + q, :] = wfull[tap, rc]

    bias_eff = bias.astype(np.float64) + np.einsum("j,ijo->o", Bconst, w)
    return (
        np.ascontiguousarray(wpk, dtype=np.float32),
        np.ascontiguousarray(bias_eff[:, None], dtype=np.float32),
    )


def _features_np(x):
    """Per-element features, fp32, matching the device computation.
    x: (..., ) -> (..., 8)"""
    x = x.astype(np.float32)
    feats = [x, x * x, (x * x) * x]
    for sc, b in ((-2.5, -1.5), (-2.5, -0.5), (2.5, -0.5), (2.5, -1.5)):
        v = np.maximum(np.float32(sc) * x + np.float32(b), np.float32(0.0))
        feats.append((v * v) * v)
    sig = 1.0 / (1.0 + np.exp(-x.astype(np.float64)))
    feats.append((x.astype(np.float64) * sig).astype(np.float32))
    return np.stack(feats, axis=-1)


def reference_sim(inputs, spline_kernel, scale_factor, bias, grid=None):
    """Host numpy simulation of the kernel math (for validation)."""
    wpk, bias_eff = _prep_weights(spline_kernel, scale_factor, bias)
    xb = inputs.astype(np.float32)
    feats = _features_np(xb).astype(np.float64)  # (B, H, W, 32, 8)
    out = np.zeros((xb.shape[0], OH, OW, FILTERS), dtype=np.float64)
    for di in range(KH):
        for dj in range(KW):
            tap = di * 3 + dj
            f = feats[:, di : di + OH, dj : dj + OW]  # (B, OH, OW, 32, 8)
            for q in range(2):
                wq = wpk[:, tap * 2 + q, :].astype(np.float64)  # (128, 128)
                # krow = rloc*32 + c, rc = q*4 + rloc
                fq = f[..., :, q * 4 : (q + 1) * 4]  # (..., 32, 4) c, rloc
                fq = np.moveaxis(fq, -1, -2).reshape(*f.shape[:3], 128)
                out += fq @ wq
    return (out + bias_eff[:, 0]).astype(np.float32)


def _build_program():
    import concourse.mybir as mybir
    from concourse import bacc
    from concourse.tile import TileContext
    from concourse.masks import make_identity

    FP = mybir.dt.float32
    FPR = mybir.dt.float32r
    BF = mybir.dt.bfloat16
    AF = mybir.ActivationFunctionType

    nc = bacc.Bacc()
    x_d = nc.dram_tensor("x", [BLOC, H, W, C], FP, kind="ExternalInput")
    w_d = nc.dram_tensor("wpk", [128, _NTAP * 2, FILTERS], FPR, kind="ExternalInput")
    b_d = nc.dram_tensor("bias_eff", [128, 1], FP, kind="ExternalInput")
    o_d = nc.dram_tensor("out", [128, OH, BLOC, OW], FP, kind="ExternalOutput")

    with TileContext(nc) as tc:
        with (
            tc.tile_pool(name="singles", bufs=1) as singles,
            tc.tile_pool(name="xp", bufs=3) as xp,
            tc.tile_pool(name="bp", bufs=3) as bp,
            tc.tile_pool(name="vp", bufs=2) as vp,
            tc.tile_pool(name="op", bufs=2) as op,
            tc.tile_pool(name="pt", bufs=4, space="PSUM") as pt,
            tc.tile_pool(name="po", bufs=2, space="PSUM") as po,
        ):
            # group-0 x loads go first so the sync DMA queue starts them
            # during boot (everything else below can overlap them)
            x4_0 = xp.tile([128, 4, C], FP, name="x4_0", tag="x4")
            for im in range(BLOC):
                src0 = x_d[im, 0:4, :, :].rearrange("r x c -> x r c")
                # image 0 on sync, image 1 on the scalar HWDGE queue: the two
                # descriptor gens run in parallel, landing x4_0 ~1.2us sooner
                # (it gates the whole feats(0) -> T(0) -> copies -> B(0) boot
                # chain)
                deng0 = nc.sync if im == 0 else nc.scalar
                deng0.dma_start(out=x4_0[im * 64 : (im + 1) * 64, :, :], in_=src0)

            ident = singles.tile([128, 128], FP)
            make_identity(nc, ident)
            identr = singles.tile([128, 128], FPR)
            nc.vector.tensor_copy(identr, ident)
            rbias = singles.tile([128, 2], FP)
            nc.gpsimd.memset(rbias[:, 0:1], -1.5)
            nc.gpsimd.memset(rbias[:, 1:2], -0.5)
            # pre-warm the Silu ACT table so its ~1.5us load happens during
            # boot (the relus moved to vector ops, so the Relu table is never
            # loaded at all)
            warm = singles.tile([128, 1], FP)
            nc.scalar.activation(warm, rbias[:, 0:1], getattr(AF, _SILU_NAME))

            # PE pre-heat: the Tensor engine's DVFS ramp means cold matmuls
            # run ~1.5x slower; burn ~5 us of zero matmuls during boot so the
            # clock is ramped when the real stream starts.  fp32 (4 cyc/col)
            # is used to get long-running instructions from few issues.
            zpre = singles.tile([128, 512], FP)
            nc.gpsimd.memset(zpre.rearrange("p a -> p a"), 0.0)
            zps = po.tile([128, 512], FP, name="zps", tag="ps")
            for i in range(3):
                nc.tensor.matmul(
                    zps, zpre[:, 0:128], zpre, start=True, stop=True
                )

            wt = singles.tile([128, _NTAP * 2, FILTERS], FPR)
            biasT = singles.tile([128, 1], FP)
            # feature-transpose buffers: [krow 128, row 64, img 2, x 64]
            bt0 = singles.tile([128, H, BLOC, 64], FPR)
            bt1 = singles.tile([128, H, BLOC, 64], FPR)
            bts = [bt0, bt1]

            # Phase FEAT (per group of 4 input rows): compute the 8 features
            # per pixel in [pixel, feature*32+c] layout.  Relus are emitted
            # before silu so the chunk-0 cube chain starts as early as
            # possible on the scalar queue.
            def phase_feat(g):
                if g == 0:
                    x4 = x4_0
                else:
                    x4 = xp.tile([128, 4, C], FP, name=f"x4_{g}", tag="x4")
                    # group 1 rides the gpsimd SWDGE queue: on the sync queue
                    # it lands ~4us late at boot and stalls the scalar-queue
                    # drain copies (and with them the first phase_b) behind
                    # silu(1)
                    deng = nc.gpsimd if g == 1 else nc.sync
                    for im in range(BLOC):
                        src = x_d[im, g * 4 : (g + 1) * 4, :, :].rearrange(
                            "r x c -> x r c"
                        )
                        deng.dma_start(
                            out=x4[im * 64 : (im + 1) * 64, :, :], in_=src
                        )
                b4 = bp.tile([128, 4, 256], FPR, name=f"b4_{g}", tag="b4")
                x2t = vp.tile([128, 4, C], FP, name=f"x2t_{g}", tag="x2t")
                V = vp.tile([128, 4, 128], FP, name=f"V_{g}", tag="V")
                V2 = vp.tile([128, 4, 128], FP, name=f"V2_{g}", tag="V2")

                # relus on VECTOR (fused mul-add then in-place max) so the
                # scalar engine's boot-time ACT-table chain never gates the
                # feature pipeline: S1 = relu(-2.5x - 1.5)^3, S2 = relu(-2.5x
                # - 0.5)^3, R3 = relu(2.5x - 0.5)^3, R4 = relu(2.5x - 1.5)^3.
                # S1 block first so the chunk-0 transpose starts early.
                AL = mybir.AluOpType
                nc.vector.tensor_scalar(V[:, :, 0:32], x4, -2.5, -1.5, AL.mult, AL.add)
                nc.vector.tensor_scalar_max(V[:, :, 0:32], V[:, :, 0:32], 0.0)
                nc.vector.tensor_mul(V2[:, :, 0:32], V[:, :, 0:32], V[:, :, 0:32])
                nc.vector.tensor_mul(b4[:, :, 96:128], V2[:, :, 0:32], V[:, :, 0:32])
                nc.vector.tensor_copy(b4[:, :, 0:32], x4)  # x
                nc.vector.tensor_mul(x2t, x4, x4)
                nc.vector.tensor_copy(b4[:, :, 32:64], x2t)  # x^2
                nc.vector.tensor_mul(b4[:, :, 64:96], x2t, x4)  # x^3
                for i, (sc, bv) in enumerate(
                    ((-2.5, -0.5), (2.5, -0.5), (2.5, -1.5)), start=1
                ):
                    nc.vector.tensor_scalar(
                        V[:, :, i * 32 : (i + 1) * 32], x4, sc, bv, AL.mult, AL.add
                    )
                nc.vector.tensor_scalar_max(V[:, :, 32:128], V[:, :, 32:128], 0.0)
                nc.vector.tensor_mul(V2[:, :, 32:128], V[:, :, 32:128], V[:, :, 32:128])
                nc.vector.tensor_mul(b4[:, :, 128:224], V2[:, :, 32:128], V[:, :, 32:128])
                nc.scalar.activation(b4[:, :, 224:256], x4, getattr(AF, _SILU_NAME))
                return b4

            # Phase T (per group): PE-transpose b4 into bt0/bt1.  The 4 rows
            # of one chunk batch into a single PSUM bank tile (start=True
            # zeroes the whole 2KB region; later rows accumulate onto
            # pending-zero bytes), then one vector copy moves all 4 rows out.
            def phase_t(g, b4):
                for q in range(2):
                    ptile = pt.tile([128, 4, 128], FPR, name=f"pt_{g}_{q}", tag="pt")
                    for r in range(4):
                        nc.tensor.matmul(
                            ptile[:, r, :],
                            b4[:, r, q * 128 : (q + 1) * 128],
                            identr,
                            is_transpose=True,
                            start=(r == 0),
                            stop=(r == 3),
                            skip_group_check=True,
                        )
                    # drain on the scalar engine: vector is loaded with the
                    # feature relus/cubes, scalar only has silu + bias now
                    dst = bts[q][:, g * 4 : (g + 1) * 4]
                    nc.scalar.copy(dst.rearrange("p r i x -> p (r i x)"),
                                   ptile.rearrange("p r x -> p (r x)"))

            # Phase B (per group of 4 output rows, N = 4*124 = 496 columns).
            # di = 0 matmuls only need rows from phase_t(og) (2 groups back),
            # so they run while the copies of phase_t(og+1)'s rows drain; the
            # di = 1, 2 matmuls then find their rows ready.  62 = 14*4 + 2*3:
            # the last two groups are 3 rows (N = 372, still >= the 256 fp32r
            # needs for 1 col/cycle) so no rows are recomputed.
            def phase_b(og):
                if og <= 13:
                    y0, nr = og * 4, 4
                elif og == 14:
                    y0, nr = 56, 3
                else:
                    y0, nr = 59, 3
                ps = po.tile([128, nr, 124], FP, name=f"ps_{og}", tag="ps")
                idx = 0
                for q in range(2):
                    for di in range(KH):
                        for dj in range(KW):
                            rhs = bts[q][:, y0 + di : y0 + di + nr, :, dj : dj + 62]
                            nc.tensor.matmul(
                                ps,
                                wt[:, (di * 3 + dj) * 2 + q, :],
                                rhs,
                                start=(idx == 0),
                                stop=(idx == 17),
                            )
                            idx += 1
                ot = op.tile([128, nr, 124], FP, name=f"ot_{og}", tag="ot")
                nc.scalar.activation(
                    ot,
                    ps,
                    AF.Identity,
                    bias=biasT[:, 0:1],
                    scale=1.0,
                )
                nc.sync.dma_start(out=o_d[:, y0 : y0 + nr, :, :], in_=ot)

            b4s = {0: phase_feat(0)}
            # Identity-table warm after phase_feat(0)'s scalar ops
            nc.scalar.activation(warm, rbias[:, 0:1], AF.Identity, bias=rbias[:, 0:1], scale=1.0)
            # The weight load is chunked on the gpsimd SWDGE queue (one big
            # SWDGE transfer would stall behind its descriptor generation)
            for wch in range(3):
                nc.gpsimd.dma_start(
                    out=wt[:, wch * 6 : (wch + 1) * 6, :],
                    in_=w_d[:, wch * 6 : (wch + 1) * 6, :],
                )
            nc.gpsimd.dma_start(out=biasT, in_=b_d[:, :])
            b4s[1] = phase_feat(1)
            phase_t(0, b4s.pop(0))
            b4s[2] = phase_feat(2)
            phase_t(1, b4s.pop(1))
            # phase_b lags phase_t by TWO groups: during pipeline fill the
            # scalar drain-copy chain (silu -> copies -> bias) hasn't
            # amortized yet, and a one-group lag leaves the PE stalling
            # ~0.8us per early group (each stall also resets the DVFS ramp)
            for g in range(2, H // 4):
                phase_t(g, b4s.pop(g))
                if g + 1 < H // 4:
                    b4s[g + 1] = phase_feat(g + 1)
                phase_b(g - 2)
            phase_b(14)
            phase_b(15)
    nc.compile()
    return nc


def _get_program():
    if "nc" not in _program_cache:
        _program_cache["nc"] = _build_program()
    return _program_cache["nc"]


def run_cores(inputs, spline_kernel, scale_factor, bias, trace=False):
    """Run the SPMD kernel on 8 cores; returns (out, BassKernelResults)."""
    from concourse.bass_utils import run_bass_kernel_spmd

    wpk, bias_eff = _prep_weights(spline_kernel, scale_factor, bias)
    x = np.ascontiguousarray(inputs, dtype=np.float32)
    in_maps = [
        {
            "x": x[i * BLOC : (i + 1) * BLOC],
            "wpk": wpk,
            "bias_eff": bias_eff,
        }
        for i in range(NCORES)
    ]
    nc = _get_program()
    res = run_bass_kernel_spmd(nc, in_maps, list(range(NCORES)), trace=trace)
    out = np.empty((B, OH, OW, FILTERS), dtype=np.float32)
    for i in range(NCORES):
        oc = res.results[i]["out"]  # [128, OH, BLOC, OW]
        out[i * BLOC : (i + 1) * BLOC] = np.transpose(oc, (2, 1, 3, 0))
    return out, res


def kernel(inputs, spline_kernel, scale_factor, bias, grid=None, **_):
    out, _res = run_cores(inputs, spline_kernel, scale_factor, bias, trace=False)
    return out
